# revision 61
# baseline (speedup 1.0000x reference)
"""Trainium2 Bass kernel for nn_Decoder_74328704024734.

LSTM decoder: 32 data-dependent steps of an LSTM cell over batch 256, each
step followed by a dense [200, 13042] projection + softmax + argmax.

Strategy (8 NeuronCores, data-parallel over the batch axis, 32 rows/core):
  - The `locations` gather over h_enc is resolved on the host (int32 input is
    known at call time); the device sees the pre-gathered, pre-transposed
    encoder slices.
  - biases are folded into the matmuls via an augmented contraction row
    (K = 200 + 1 ones-row), so K chunks are 128 + 73.
  - LSTM gate math runs batch-major [32, 800]; sigmoid(x) is computed as
    (1 + tanh(x/2))/2 so the whole kernel only uses the `exp_and_others`
    activation table set (no table-switch thrash with the softmax exp).
  - h_t is transposed on the PE each step into a persistent [201, 1024]
    "HT" buffer (8 tiles of 4 steps each) that doubles as the lhsT for both
    the next step's x @ Wx matmul and the dense projection.
  - Dense phase per 128-row chunk (4 steps x 32 batch): f32r matmuls into
    [128,768] PSUM tiles (f32r streams 1 col/cycle vs 4 for strict fp32),
    exp on ACT (PSUM->SBUF with per-instruction sum accumulators giving the
    softmax denominator for free), normalization split between DVE and the
    otherwise-idle GPSIMD via tensor_scalar with a per-partition reciprocal,
    then 1536-column DMA stores.
  - game_orders needs exact-fp32 ordering (f32r's ~1e-4 rounding flips a
    handful of near-tied argmaxes), so the host takes the top-8 candidates
    per row from the device probs and re-ranks just those with exact-fp32
    logits from a cheap (~5 GFLOP) numpy recompute of the small LSTM chain.
"""

import sys

sys.path.insert(0, "/opt/trn_rl_repo")

import numpy as np

N_CORES = 8
N_PHASES = 256
N_PROV = 81
H_ENC = 120
EMB = 80
LSTM = 200
VOCAB = 13042
N_LOC = 32
GO_IDX = 1

B = N_PHASES // N_CORES  # 32 batch rows per core
S = N_LOC                # 32 steps
ROWS = S * B             # 1024 (step, batch) rows per core
KA = 128                 # first K chunk of the augmented 201-row contraction
KB = 73                  # second K chunk (72 weight rows + 1 ones row)
Z = 4 * LSTM             # 800

_CACHE = {}


def _build_nc():
    import concourse.tile as tile
    from concourse import bacc, mybir
    from concourse.alu_op_type import AluOpType

    f32 = mybir.dt.float32
    f32r = mybir.dt.float32r
    Tanh = mybir.ActivationFunctionType.Tanh
    Exp = mybir.ActivationFunctionType.Exp
    Prelu = mybir.ActivationFunctionType.Prelu
    AX = mybir.AxisListType.X

    nc = bacc.Bacc(
        "TRN2", target_bir_lowering=False, debug=False, num_devices=N_CORES
    )

    dram = {}

    def din(name, shape, dt=f32):
        dram[name] = nc.dram_tensor(name, shape, dt, kind="ExternalInput").ap()

    # float32r tensors are plain fp32 bits on the host; the PE runs them in
    # the fast "replicated" matmul mode (1 col/cycle vs 4 for strict fp32).
    din("enc_t", [H_ENC, ROWS], f32r)  # (d, s*32+b) encoder, transposed
    din("ench", [S, B, H_ENC])         # (s, b, d) encoder, batch-major
    # packed LSTM weights: one DMA each instead of five
    # wpa = [wxa | x0a | ident], wpb = [wxb | x0b]
    din("wpa", [KA, Z + B + B], f32r)
    din("wpb", [KB, Z + B], f32r)
    din("wh", [H_ENC, Z], f32r)        # lstm_Wh rows 80:200 (0:80 hit zeros)
    # dense weights split in 4 column chunks so the first dense chunk can
    # start before the whole 10.5 MB lands
    DWSPLIT = (4096, 4096, 4096, 754)
    for k in range(4):
        din(f"dwa{k}", [KA, DWSPLIT[k]], f32r)
        din(f"dwb{k}", [KB, DWSPLIT[k]], f32r)
    probs = nc.dram_tensor(
        "probs", [S, B, VOCAB], f32, kind="ExternalOutput"
    ).ap()

    with tile.TileContext(nc) as tc:
        with (
            tc.tile_pool(name="wpool", bufs=1) as wpool,
            tc.tile_pool(name="stage", bufs=12) as spool,
            tc.tile_pool(name="small", bufs=2) as sm,
            tc.tile_pool(name="encp", bufs=8) as encp,
            tc.tile_pool(name="zps", bufs=1, space="PSUM") as zpool,
            tc.tile_pool(name="mmps", bufs=2, space="PSUM") as mmpool,
            tc.tile_pool(name="tpps", bufs=1, space="PSUM") as tppool,
        ):
            # ---- persistent SBUF tiles ----
            wpa = wpool.tile([KA, Z + 2 * B], f32r, tag="wpa")
            wpb = wpool.tile([KB, Z + B], f32r, tag="wpb")
            wxa, x0a = wpa[:, 0:Z], wpa[:, Z:Z + B]
            ident = wpa[0:B, Z + B:Z + 2 * B].bitcast(f32)
            wxb, x0b = wpb[:, 0:Z], wpb[:, Z:Z + B]
            wh = wpool.tile([H_ENC, Z], f32r, tag="wh")
            enc_t = wpool.tile([H_ENC, ROWS], f32r, tag="enc_t")
            dwa = [
                wpool.tile([KA, DWSPLIT[k]], f32r, tag=f"dwa{k}",
                           name=f"dwa{k}") for k in range(4)
            ]
            dwb = [
                wpool.tile([KB, DWSPLIT[k]], f32r, tag=f"dwb{k}",
                           name=f"dwb{k}") for k in range(4)
            ]
            ht0 = [
                wpool.tile([KA, 4 * B], f32r, tag=f"ht0_{m}", name=f"ht0_{m}")
                for m in range(8)
            ]
            ht1 = [
                wpool.tile([KB, 4 * B], f32r, tag=f"ht1_{m}", name=f"ht1_{m}")
                for m in range(8)
            ]

            # ---- input DMAs: small/LSTM-critical first, big dense weights last
            nc.sync.dma_start(wpa[:], dram["wpa"][:])
            nc.sync.dma_start(wpb[:], dram["wpb"][:])
            nc.sync.dma_start(wh[:], dram["wh"][:])
            nc.sync.dma_start(enc_t[:], dram["enc_t"][:])
            for k in range(4):
                nc.sync.dma_start(dwa[k][:], dram[f"dwa{k}"][:])
                nc.sync.dma_start(dwb[k][:], dram[f"dwb{k}"][:])

            # h carries a constant 1.0 in col 200 so the second transpose
            # lands the augmentation ones-row at HT1 partition 72 directly
            h_t = sm.tile([B, 201], f32, tag="h", bufs=1)
            nc.vector.memset(h_t[:, 200:201], 1.0)

            # ---- LSTM chain ----
            def lstm_step(s):
                # x-side lhsT = transposed h of the previous step (aug. ones row)
                if s == 0:
                    xa, xb = x0a, x0b
                else:
                    m, c = (s - 1) // 4, ((s - 1) % 4) * B
                    xa = ht0[m][:, c:c + B]
                    xb = ht1[m][:, c:c + B]

                es = encp.tile([B, H_ENC], f32, tag="es")
                nc.sync.dma_start(es[:], dram["ench"][s])

                zp = zpool.tile([B, Z], f32, tag="z")
                # matmul outputs must not straddle a 2KB PSUM bank: 512 | 288
                for n0, n1 in ((0, 512), (512, Z)):
                    # enc-side first: independent of h, can prefetch into PSUM
                    nc.tensor.matmul(
                        zp[:, n0:n1], enc_t[:, s * B:(s + 1) * B],
                        wh[:, n0:n1], start=True, stop=False,
                    )
                    nc.tensor.matmul(
                        zp[:, n0:n1], xa, wxa[:, n0:n1],
                        start=False, stop=False,
                    )
                    nc.tensor.matmul(
                        zp[:, n0:n1], xb, wxb[:, n0:n1],
                        start=False, stop=True,
                    )

                # gates: z columns are host-permuted to [zi | zf | zo | zg].
                # tanh is split at the z-chunk boundary so it can start as
                # soon as the first 512-col accumulation group completes.
                # sigmoid(x) = (1 + tanh(x/2))/2; we track c' = 2c and fold
                # the 1/2 factors into Prelu input scales, so each gate-mul
                # is a single fused (tanh + 1) * y scalar_tensor_tensor op.
                t_ifo = sm.tile([B, 600], f32, tag="t_ifo", bufs=1)
                nc.scalar.activation(t_ifo[:], zp[:, 0:600], Tanh, scale=0.5)
                # g = lrelu(zg) = max(zg, 0.2 zg) on DVE
                g1 = sm.tile([B, 200], f32, tag="g1", bufs=1)
                nc.vector.tensor_scalar_mul(g1[:], zp[:, 600:800], 0.2)
                g = sm.tile([B, 200], f32, tag="g", bufs=1)
                nc.vector.scalar_tensor_tensor(
                    g[:], zp[:, 600:800], 1.0, g1[:],
                    op0=AluOpType.mult, op1=AluOpType.max,
                )

                # c' = 2c = (1+t_i)*g  (+ (1+t_f)*enc on cols 80:200)
                c = sm.tile([B, 200], f32, tag="c", bufs=1)
                nc.vector.scalar_tensor_tensor(
                    c[:], t_ifo[:, 0:200], 1.0, g[:],
                    op0=AluOpType.add, op1=AluOpType.mult,
                )
                v = sm.tile([B, H_ENC], f32, tag="v", bufs=1)
                nc.vector.scalar_tensor_tensor(
                    v[:], t_ifo[:, 280:400], 1.0, es[:],
                    op0=AluOpType.add, op1=AluOpType.mult,
                )
                nc.vector.tensor_add(c[:, 80:200], c[:, 80:200], v[:])

                # cth = lrelu(c)/2 = max(0.25 c', 0.05 c'); h = (1+t_o)*cth
                c1 = sm.tile([B, 200], f32, tag="c1", bufs=1)
                nc.vector.tensor_scalar_mul(c1[:], c[:], 0.05)
                cth = sm.tile([B, 200], f32, tag="cth", bufs=1)
                nc.vector.scalar_tensor_tensor(
                    cth[:], c[:], 0.25, c1[:],
                    op0=AluOpType.mult, op1=AluOpType.max,
                )
                h = h_t
                nc.vector.scalar_tensor_tensor(
                    h[:, 0:200], t_ifo[:, 400:600], 1.0, cth[:],
                    op0=AluOpType.add, op1=AluOpType.mult,
                )

                # transpose h into HT group s+1 (tile s//4, col (s%4)*B)
                mt, ct0 = s // 4, (s % 4) * B
                pa = tppool.tile([KA, B], f32, tag="pa")
                nc.tensor.transpose(pa[:], h[:, 0:128], ident)
                pb = tppool.tile([KB, B], f32, tag="pb")
                nc.tensor.transpose(pb[:], h[:, 128:201], ident)
                nc.vector.tensor_copy(ht0[mt][:, ct0:ct0 + B], pa[:])
                nc.vector.tensor_copy(ht1[mt][:, ct0:ct0 + B], pb[:])

            for s in range(S):
                lstm_step(s)

            # ---- dense + softmax, one 128-row chunk (4 steps) at a time ----
            out_r = probs.rearrange("s b v -> (s b) v")

            def dense_chunk(m):
                zacc = sm.tile([128, 32], f32, tag="zacc")
                stages = []
                nacc = 0
                for blk in range(9):
                    c0 = blk * 1536
                    w = min(1536, VOCAB - c0)
                    st = spool.tile([128, 1536], f32, tag="stage")
                    stages.append((st, c0, w))
                    for half in range(0, w, 768):
                        p0 = c0 + half
                        pw = min(768, VOCAB - p0)
                        pt = mmpool.tile([128, 768], f32, tag="mm")
                        for nn in range(0, pw, 256):
                            nw = min(256, pw - nn)
                            col = p0 + nn
                            k = min(col // 4096, 3)
                            rel = col - 4096 * k
                            nc.tensor.matmul(
                                pt[:, nn:nn + nw], ht0[m],
                                dwa[k][:, rel:rel + nw],
                                start=True, stop=False,
                            )
                            nc.tensor.matmul(
                                pt[:, nn:nn + nw], ht1[m],
                                dwb[k][:, rel:rel + nw],
                                start=False, stop=True,
                            )
                        nc.scalar.activation(
                            st[:, half:half + pw], pt[:, 0:pw], Exp,
                            accum_out=zacc[:, nacc:nacc + 1],
                        )
                        nacc += 1
                zsum = sm.tile([128, 1], f32, tag="zsum")
                nc.vector.reduce_sum(zsum[:], zacc[:, 0:nacc], axis=AX)
                rec = sm.tile([128, 1], f32, tag="rec")
                nc.vector.reciprocal(rec[:], zsum[:])
                for bi2, (st, c0, w) in enumerate(stages):
                    # chain is over by m7: use the faster DVE exclusively
                    eng = nc.vector if (m == 7 or bi2 % 3 == 0) else nc.gpsimd
                    eng.tensor_scalar_mul(st[:, 0:w], st[:, 0:w], rec[:])
                    nc.sync.dma_start(
                        out_r[4 * B * m:4 * B * (m + 1), c0:c0 + w], st[:, 0:w]
                    )

            for m in range(8):
                dense_chunk(m)

    nc.compile()
    return nc


def _get_nc():
    if "nc" not in _CACHE:
        _CACHE["nc"] = _build_nc()
    return _CACHE["nc"]


def kernel(
    h_enc, emb_table, lstm_Wx, lstm_Wh, lstm_b, dense_W, dense_b, locations
):
    from concourse.bass_utils import run_bass_kernel_spmd

    h_enc = np.asarray(h_enc, dtype=np.float32)
    emb_table = np.asarray(emb_table, dtype=np.float32)
    lstm_Wx = np.asarray(lstm_Wx, dtype=np.float32)
    lstm_Wh = np.asarray(lstm_Wh, dtype=np.float32)
    lstm_b = np.asarray(lstm_b, dtype=np.float32)
    dense_W = np.asarray(dense_W, dtype=np.float32)
    dense_b = np.asarray(dense_b, dtype=np.float32)
    locations = np.asarray(locations)

    # host-side prep (cheap, data-layout only)
    sel = h_enc[:, locations, :]                      # [256, 32, 120] (b, s, d)

    x0 = np.zeros((KA + KB, B), dtype=np.float32)     # [201, 32]
    x0[0:EMB, :] = emb_table[GO_IDX][:, None]         # go embedding, bcast
    x0[200, :] = 1.0                                  # augmentation ones-row

    wx_aug = np.concatenate([lstm_Wx, lstm_b[None, :]], axis=0)   # [201, 800]
    dw_aug = np.concatenate([dense_W, dense_b[None, :]], axis=0)  # [201, 13042]
    # permute gate columns [zi|zf|zg|zo] -> [zi|zf|zo|zg] so the kernel can
    # tanh all three sigmoid gates in one instruction
    perm = np.concatenate(
        [np.arange(0, 400), np.arange(600, 800), np.arange(400, 600)]
    )
    wx_aug = np.ascontiguousarray(wx_aug[:, perm])

    wpa = np.zeros((KA, 800 + 2 * B), dtype=np.float32)
    wpa[:, 0:800] = wx_aug[0:KA]
    wpa[:, 800:800 + B] = x0[0:KA]
    wpa[0:B, 800 + B:800 + 2 * B] = np.eye(B, dtype=np.float32)
    wpb = np.zeros((KB, 800 + B), dtype=np.float32)
    wpb[:, 0:800] = wx_aug[KA:]
    wpb[:, 800:800 + B] = x0[KA:]
    common = {
        "wpa": wpa,
        "wpb": wpb,
        "wh": np.ascontiguousarray(lstm_Wh[EMB:, :][:, perm]),
        **{f"dwa{k}": np.ascontiguousarray(
            dw_aug[0:KA, sum((4096, 4096, 4096, 754)[:k]):
                   sum((4096, 4096, 4096, 754)[:k + 1])])
           for k in range(4)},
        **{f"dwb{k}": np.ascontiguousarray(
            dw_aug[KA:, sum((4096, 4096, 4096, 754)[:k]):
                   sum((4096, 4096, 4096, 754)[:k + 1])])
           for k in range(4)},
    }
    in_maps = []
    for c in range(N_CORES):
        enc_c = sel[c * B:(c + 1) * B]                # [32(b), 32(s), 120]
        m = dict(common)
        # (d, s*32+b) for matmul lhsT
        m["enc_t"] = np.ascontiguousarray(
            enc_c.transpose(2, 1, 0).reshape(H_ENC, ROWS)
        )
        # (s, b, d) for the elementwise c-update
        m["ench"] = np.ascontiguousarray(enc_c.transpose(1, 0, 2))
        in_maps.append(m)

    nc = _get_nc()
    _CACHE["last_in_maps"] = in_maps
    res = run_bass_kernel_spmd(nc, in_maps, list(range(N_CORES)))

    probs = np.empty((S, N_PHASES, VOCAB), dtype=np.float32)
    for c in range(N_CORES):
        probs[:, c * B:(c + 1) * B, :] = res.results[c]["probs"]

    orders = _exact_orders(
        probs, h_enc, emb_table, lstm_Wx, lstm_Wh, lstm_b, dense_W, dense_b,
        locations,
    )
    return orders, probs


def _exact_orders(
    probs, h_enc, emb_table, lstm_Wx, lstm_Wh, lstm_b, dense_W, dense_b,
    locations,
):
    """Device probs carry the f32r matmul rounding (~1e-4); near-tied top-2
    logits can flip the argmax vs the exact-fp32 reference. Refine: take the
    device top-8 candidates per row and rank just those by exact-fp32 logits
    (recomputing the small LSTM chain on the host, ~5 GFLOP)."""
    n = h_enc.shape[0]

    def sig(x):
        return 1.0 / (1.0 + np.exp(-x))

    def lrelu(x):
        return np.where(x >= 0, x, np.float32(0.2) * x)

    x = np.zeros((n, LSTM), dtype=np.float32)
    x[:, 0:EMB] = emb_table[GO_IDX]
    hs = np.empty((S, n, LSTM), dtype=np.float32)
    state = np.zeros((n, LSTM), dtype=np.float32)
    for s in range(S):
        state[:, EMB:] = h_enc[:, locations[s], :]
        z = x @ lstm_Wx + state @ lstm_Wh + lstm_b
        zi, zf, zg, zo = np.split(z, 4, axis=1)
        c = sig(zf) * state + sig(zi) * lrelu(zg)
        x = sig(zo) * lrelu(c)
        hs[s] = x

    orders = np.empty((S, n), dtype=np.int32)
    for s in range(S):
        cand = np.argpartition(probs[s], -8, axis=-1)[:, -8:]  # [n, 8]
        cand.sort(axis=1)  # ascending: first-index tie rule like argmax
        wc = dense_W[:, cand.reshape(-1)].reshape(LSTM, n, 8)
        lg = np.einsum("rd,drk->rk", hs[s], wc) + dense_b[cand]
        orders[s] = cand[np.arange(n), np.argmax(lg, axis=1)]
    return orders
